# revision 16
# baseline (speedup 1.0000x reference)
"""Trainium2 Bass kernel for nn_Attention (B=4, L=1024, D=768, H=12, DH=64).

Reference per (batch b, head n):
    A = q_n^T k_n                [D, D]
    scores = x A x^T             [L, L]
    S = softmax(scores, -1)
    V = v_n @ x_b^T              [DH, L]
    out[b, l, n*DH+e] = sum_m S[l, m] V[e, m] / sum_m S[l, m]

Sharding: 48 (b, n) units over 8 cores; core c owns batch pair c//4 and
head triple c%4.  A is computed once per head, reused for both batches;
heads 1,2 of each triple are split column-wise across the core pair and
exchanged with a 2-rank AllGather that overlaps head-0 compute.

fp8 acceleration: A is decomposed as abar*J + Atil (abar = mean, J =
ones).  q, k are uniform-positive so A is concentrated around its mean
-- quantizing it directly to e4m3 biases the dominant rank-1 logit term
abar*S(x)S(x)^T and blows the error budget.  Instead the zero-centered
Atil runs through fp8e4 DoubleRow matmuls (2 contraction chunks per
instruction, both operands fp8) for A, WT = Atil^T x^T and the scores,
while the exact rank-1 term is injected per scores-psum with a K=1
float32r matmul (lhsT = abar*2^15*S row, rhs = S row, host-computed in
f64).  The V/R path stays bf16 (V errors land directly in the output).

Scales (e4m3 max is 240 on TRN -- values above become Inf):
    q8, k8, v-host   x512
    A psum           = A_q8 * 2^18;  at = (psum - mean)*2^2 = Atil*2^20
    WT psum          = Atil*2^20 x;  wt = psum * 2^-5 = Wtil*2^15
    scores psum      = 2^15 * (x Atil x^T + abar S S^T);  exp scale 2^-15
"""

from contextlib import ExitStack

import ml_dtypes
import numpy as np

import concourse.tile as tile
from concourse import bacc, mybir
from concourse.bass import ts, ds
from concourse.bass_utils import run_bass_kernel_spmd

# If BASS_TRACE is set in an environment that lacks antenv.axon_hooks,
# run_bass_kernel_spmd's trace path would fail on import; register a
# fallback holder so tracing degrades gracefully instead.
try:
    import antenv.axon_hooks  # noqa: F401
except Exception:  # pragma: no cover
    import sys
    import types

    import antenv

    _m = types.ModuleType("antenv.axon_hooks")
    _m._hook = None
    _m.set_axon_ntff_profile_hook = lambda h: setattr(_m, "_hook", h)
    _m.get_axon_ntff_profile_hook = lambda: _m._hook
    sys.modules["antenv.axon_hooks"] = _m
    antenv.axon_hooks = _m

B, L, D, H = 4, 1024, 768, 12
DH = D // H          # 64
HPC = 3              # heads per core
BPC = 2              # batches per core
N_CORES = 8
DC = D // 128        # 6 chunks of the contraction/feature dim
JP = DC // 2         # 3 DoubleRow chunk-pairs
LB = L // 128        # 8 l-blocks / m-blocks
DHA = DH + 1         # 65: head slice width in vt_aug (ones column at 64)
DHALF = D // 2       # 384: A-half width for the pairwise exchange
F32 = mybir.dt.float32
F32R = mybir.dt.float32r
BF16 = mybir.dt.bfloat16
F8 = mybir.dt.float8e4
DR = mybir.MatmulPerfMode.DoubleRow
PAIR_GROUPS = [[0, 4], [1, 5], [2, 6], [3, 7]]

SQK = 512.0          # host scale on q, k
A_SC = 4.0           # at = (A_psum - mean) * 2^2  -> Atil * 2^20
W_SC = 2.0**-5       # wt = WT_psum * 2^-5         -> Wtil * 2^15
S_SC = 2.0**15       # scores psum scale; exp uses 1/S_SC
N_WARMUP = 28        # dummy matmuls to lift the HAM clock gate early

_COMPILED = None


def _build():
    nc = bacc.Bacc(
        "TRN2",
        target_bir_lowering=False,
        debug=False,
        enable_asserts=False,
        num_devices=N_CORES,
    )
    xT8_ext = nc.dram_tensor("xT8", [BPC, D, L], F8, kind="ExternalInput").ap()
    xTb_ext = nc.dram_tensor("xTb", [BPC, D, L], BF16, kind="ExternalInput").ap()
    q3_ext = nc.dram_tensor("q3", [HPC, D, D], F8, kind="ExternalInput").ap()
    k0_ext = nc.dram_tensor("k0", [D, D], F8, kind="ExternalInput").ap()
    kh_ext = nc.dram_tensor("kh", [2, D, D], F8, kind="ExternalInput").ap()
    vT3_ext = nc.dram_tensor("vT3", [D, HPC * DH], BF16, kind="ExternalInput").ap()
    sbc_ext = nc.dram_tensor("sbc", [BPC, 128, L], BF16, kind="ExternalInput").ap()
    scol_ext = nc.dram_tensor("scol", [BPC, 128, LB], F32, kind="ExternalInput").ap()
    abars_ext = nc.dram_tensor("abars", [128, HPC], F32, kind="ExternalInput").ap()
    cneg_ext = nc.dram_tensor("cneg", [128, HPC], F32, kind="ExternalInput").ap()
    out_ext = nc.dram_tensor(
        "out_r", [BPC, L, HPC * DH], F32, kind="ExternalOutput"
    ).ap()

    with tile.TileContext(nc) as tc, ExitStack() as ctx:
        xt_pool = ctx.enter_context(tc.tile_pool(name="xt", bufs=1))
        vt3_pool = ctx.enter_context(tc.tile_pool(name="vt3", bufs=1))
        vt_pool = ctx.enter_context(tc.tile_pool(name="vt", bufs=1))
        ss_pool = ctx.enter_context(tc.tile_pool(name="ss", bufs=1))
        qk_pool = ctx.enter_context(tc.tile_pool(name="qk", bufs=1))
        a_pool = ctx.enter_context(tc.tile_pool(name="a", bufs=1))
        wt_pool = ctx.enter_context(tc.tile_pool(name="wt", bufs=2))
        pt_pool = ctx.enter_context(tc.tile_pool(name="pt", bufs=2))
        soft_pool = ctx.enter_context(tc.tile_pool(name="soft", bufs=2))
        out_pool = ctx.enter_context(tc.tile_pool(name="outp", bufs=1))
        sm_pool = ctx.enter_context(tc.tile_pool(name="sm", bufs=1))
        dram_pool = ctx.enter_context(tc.tile_pool(name="dram", bufs=1, space="DRAM"))
        ps_p = ctx.enter_context(tc.tile_pool(name="ps_p", bufs=2, space="PSUM"))
        ps_s = ctx.enter_context(tc.tile_pool(name="ps_s", bufs=4, space="PSUM"))
        ps_r = ctx.enter_context(tc.tile_pool(name="ps_r", bufs=2, space="PSUM"))

        # ---------- HAM warmup: keep the PE busy from t=0 so the clock
        # gate lifts (~3.4us) before the real matmul stream begins.
        dummy = sm_pool.tile([128, 128], F8, tag="dummy")
        nc.gpsimd.memset(dummy[:], 0.0)
        for w in range(N_WARMUP):
            pw = ps_p.tile([128, 512], F32, tag=ps_p.name)
            nc.tensor.matmul(pw[:, :128], dummy[:], dummy[:], start=True, stop=True)

        # ---------- loads ----------
        def load3d(pool, tag, dram2d, width, dtype):
            t = pool.tile([128, DC, width], dtype, tag=tag)
            nc.sync.dma_start(t[:], dram2d.rearrange("(c p) w -> p c w", p=128))
            return t

        q_all = [None, None, None]
        q_all[0] = load3d(qk_pool, "q0", q3_ext[0], D, F8)
        k0_sb = load3d(qk_pool, "k0", k0_ext[:], D, F8)
        cneg_sb = sm_pool.tile([128, HPC], F32, tag="cneg")
        nc.sync.dma_start(cneg_sb[:], cneg_ext[:])
        abars_sb = sm_pool.tile([128, HPC], F32, tag="abars")
        nc.sync.dma_start(abars_sb[:], abars_ext[:])
        sbc_t = [None, None]
        scol_t = [None, None]
        for bi in range(BPC):
            t_sbc = sm_pool.tile([128, L], BF16, tag=f"sbc{bi}")
            nc.sync.dma_start(t_sbc[:], sbc_ext[bi])
            sbc_t[bi] = t_sbc
            t_scol = sm_pool.tile([128, LB], F32, tag=f"scol{bi}")
            nc.sync.dma_start(t_scol[:], scol_ext[bi])
            scol_t[bi] = t_scol
        xt8 = [None, None]
        xtb = [None, None]
        xt8[0] = load3d(xt_pool, "x8_0", xT8_ext[0], L, F8)
        kh_sb = [None, None]
        for h in (1, 2):
            q_all[h] = load3d(qk_pool, f"q{h}", q3_ext[h], D, F8)
            kh_sb[h - 1] = load3d(qk_pool, f"kh{h}", kh_ext[h - 1], D, F8)
        xtb[0] = load3d(xt_pool, "xb_0", xTb_ext[0], L, BF16)
        vt3 = load3d(vt3_pool, "vt3", vT3_ext[:], HPC * DH, BF16)
        xt8[1] = load3d(xt_pool, "x8_1", xT8_ext[1], L, F8)
        xtb[1] = load3d(xt_pool, "xb_1", xTb_ext[1], L, BF16)

        # at[h][p, j, d'] = Atil[128j+p, d'] * 2^20 in fp8
        at = [
            a_pool.tile([128, DC, D], F8, tag=f"at{h}", name=f"at{h}")
            for h in range(HPC)
        ]

        # ---- Atil per head, fully local.  Head 0 runs in the cold-clock
        # startup window (it gates unit 0); heads 1, 2 are deferred into
        # the warm steady state just before their first consumer unit ----
        def build_at(h):
            k_src = k0_sb if h == 0 else kh_sb[h - 1]
            for i in range(DC):
                for n in range(2):
                    p = ps_p.tile([128, 512], F32, tag=ps_p.name)
                    for j in range(JP):
                        nc.tensor.matmul(
                            p[:, :DHALF],
                            q_all[h][:, 2 * j : 2 * j + 2, ts(i, 128)],
                            k_src[:, 2 * j : 2 * j + 2, ds(n * DHALF, DHALF)],
                            start=(j == 0),
                            stop=(j == JP - 1),
                            perf_mode=DR,
                        )
                    nc.scalar.activation(
                        at[h][:, i, ds(n * DHALF, DHALF)],
                        p[:, :DHALF],
                        mybir.ActivationFunctionType.Identity,
                        bias=cneg_sb[:, h : h + 1],
                        scale=A_SC,
                    )

        build_at(0)

        # ---------- VT_aug projection per batch (bf16, as V errors are
        # first-order in the output) ----------
        vt = [None, None]

        def build_vt(bi):
            tiles = []
            for j in range(LB):
                p = ps_p.tile([128, 512], F32, tag=ps_p.name)
                for i in range(DC):
                    nc.tensor.matmul(
                        p[:, : HPC * DH],
                        xtb[bi][:, i, ts(j, 128)],
                        vt3[:, i, :],
                        start=(i == 0),
                        stop=(i == DC - 1),
                    )
                t = vt_pool.tile([128, HPC * DHA], BF16, tag=f"vt{bi}_{j}")
                nc.gpsimd.memset(t[:], 1.0)
                t3 = t[:].rearrange("p (h c) -> p h c", h=HPC)
                p3 = p[:, : HPC * DH].rearrange("p (h c) -> p h c", h=HPC)
                nc.vector.tensor_copy(t3[:, :, :DH], p3[:])
                tiles.append(t)
            vt[bi] = tiles

        # SS[bi][mj][p, l] = S_b[128*mj+p] * S_b[l]  (unscaled rank-1 base)
        ss_t = [[None] * LB for _ in range(BPC)]

        def build_ss(bi):
            for mj in range(LB):
                t = ss_pool.tile(
                    [128, L], BF16, tag=f"ss{bi}_{mj}", name=f"ss{bi}_{mj}"
                )
                nc.vector.tensor_scalar(
                    t[:],
                    sbc_t[bi][:],
                    scol_t[bi][:, mj : mj + 1],
                    None,
                    op0=mybir.AluOpType.mult,
                )
                ss_t[bi][mj] = t

        build_vt(0)
        build_ss(0)

        # out accumulators: per batch, one [128, 192] f32 tile per l-block
        out_sb = [[], []]
        for bi in range(BPC):
            for lb in range(LB):
                ot = out_pool.tile([128, HPC * DH], F32, tag=f"out{bi}_{lb}")
                out_sb[bi].append(ot)

        for h in range(HPC):
            for bi in range(BPC):
                if bi == 0 and h > 0:
                    build_at(h)
                # ---- WT[d', l] = sum_d Atil[d, d'] xT[d, l] * 2^20 ----
                wt = wt_pool.tile([128, DC, L], F8, tag="wt")
                for i in range(DC):
                    for n in range(2):
                        p = ps_p.tile([128, 512], F32, tag=ps_p.name)
                        for j in range(JP):
                            nc.tensor.matmul(
                                p[:],
                                at[h][:, 2 * j : 2 * j + 2, ts(i, 128)],
                                xt8[bi][:, 2 * j : 2 * j + 2, ts(n, 512)],
                                start=(j == 0),
                                stop=(j == JP - 1),
                                perf_mode=DR,
                            )
                        nc.vector.tensor_scalar_mul(
                            wt[:, i, ts(n, 512)], p[:], W_SC
                        )

                if h == 0 and bi == 1:
                    build_vt(1)
                    build_ss(1)

                # ---- scoresT + rank-1 + exp + R, pipelined per (mj, n).
                # R accumulates into two held psum tiles (4 x 65 cols
                # each) across all mj so no dense R block ever forms ----
                racc = [
                    ps_r.tile([128, 512], F32, tag=ps_r.name, name=f"racc{g}")
                    for g in range(2)
                ]
                for mj in range(LB):
                    pt = pt_pool.tile([128, L], BF16, tag=f"pt{mj}")
                    for n in range(2):
                        p = ps_s.tile([128, 512], F32, tag="ps_s")
                        for j in range(JP):
                            nc.tensor.matmul(
                                p[:],
                                xt8[bi][:, 2 * j : 2 * j + 2, ts(mj, 128)],
                                wt[:, 2 * j : 2 * j + 2, ts(n, 512)],
                                start=(j == 0),
                                stop=(j == JP - 1),
                                perf_mode=DR,
                            )
                        nc.vector.scalar_tensor_tensor(
                            p[:],
                            ss_t[bi][mj][:, ts(n, 512)],
                            abars_sb[:, h : h + 1],
                            p[:],
                            op0=mybir.AluOpType.mult,
                            op1=mybir.AluOpType.add,
                        )
                        nc.scalar.activation(
                            pt[:, ts(n, 512)],
                            p[:],
                            mybir.ActivationFunctionType.Exp,
                            scale=1.0 / S_SC,
                        )
                        for lb in range(4 * n, 4 * n + 4):
                            # PSUM start=True zeroes a whole 2KB bank
                            # (ZERO_REGION granularity), so only the first
                            # slice per racc bank may issue it; the rest
                            # accumulate onto the pending-zeroed bank.
                            nc.tensor.matmul(
                                racc[lb // 4][:, ds(DHA * (lb % 4), DHA)],
                                pt[:, ts(lb, 128)],
                                vt[bi][mj][:, ds(DHA * h, DHA)],
                                start=(mj == 0 and lb % 4 == 0),
                                stop=(mj == LB - 1),
                                skip_group_check=True,
                            )
                # ---- normalize + (final head) out DMA ----
                for lb in range(LB):
                    pr = racc[lb // 4][:, ds(DHA * (lb % 4), DHA)]
                    recip = soft_pool.tile([128, 1], F32, tag="recip")
                    nc.vector.reciprocal(recip[:], pr[:, DH : DH + 1])
                    nc.vector.tensor_scalar_mul(
                        out_sb[bi][lb][:, ts(h, DH)], pr[:, :DH], recip[:]
                    )
                    if h == HPC - 1:
                        nc.sync.dma_start(
                            out_ext[bi, ts(lb, 128), :], out_sb[bi][lb][:]
                        )

    nc.compile()
    return nc


def kernel(x, k, q, v):
    global _COMPILED
    if _COMPILED is None:
        _COMPILED = _build()

    x = np.ascontiguousarray(x, dtype=np.float32)
    k = np.ascontiguousarray(k, dtype=np.float32)
    q = np.ascontiguousarray(q, dtype=np.float32)
    v = np.ascontiguousarray(v, dtype=np.float32)

    bf = ml_dtypes.bfloat16
    f8 = ml_dtypes.float8_e4m3
    xT = x.transpose(0, 2, 1)              # [B, D, L]
    xT8 = xT.astype(f8)
    xTb = xT.astype(bf)
    q8 = (q * SQK).astype(f8)
    k8 = (k * SQK).astype(f8)
    vb = v.transpose(2, 0, 1).astype(bf)   # [D, H, DH]

    # exact rank-1 pieces (f64): abar = mean(q^T k), S = row sums of x
    q64 = q.astype(np.float64)
    k64 = k.astype(np.float64)
    abar = (q64.sum(axis=2) * k64.sum(axis=2)).sum(axis=1) / (D * D)  # [H]
    S = x.astype(np.float64).sum(axis=2)                              # [B, L]
    # mean of the *quantized* A (what the device must subtract)
    q8f = q8.astype(np.float64)
    k8f = k8.astype(np.float64)
    c_ps = (q8f.sum(axis=2) * k8f.sum(axis=2)).sum(axis=1) / (D * D)  # [H]
    sbc = np.broadcast_to(S[:, None, :], (B, 128, L)).astype(bf)      # [B, 128, L]
    scol = S.reshape(B, LB, 128).transpose(0, 2, 1).astype(np.float32)  # [B, 128, LB]

    in_maps = []
    for c in range(N_CORES):
        bp, t = c // 4, c % 4
        hs = slice(HPC * t, HPC * (t + 1))
        h0 = HPC * t
        bsl = slice(BPC * bp, BPC * (bp + 1))
        in_maps.append(
            {
                "xT8": np.ascontiguousarray(xT8[bsl]),
                "xTb": np.ascontiguousarray(xTb[bsl]),
                "q3": np.ascontiguousarray(q8[hs]),
                "k0": np.ascontiguousarray(k8[h0]),
                "kh": np.ascontiguousarray(k8[h0 + 1 : h0 + 3]),
                "vT3": np.ascontiguousarray(vb[:, hs].reshape(D, HPC * DH)),
                "sbc": np.ascontiguousarray(sbc[bsl]),
                "scol": np.ascontiguousarray(scol[bsl]),
                "abars": np.ascontiguousarray(
                    np.broadcast_to(
                        (abar[hs] * S_SC).astype(np.float32)[None, :], (128, HPC)
                    )
                ),
                "cneg": np.ascontiguousarray(
                    np.broadcast_to(
                        (-A_SC * c_ps[hs]).astype(np.float32)[None, :], (128, HPC)
                    )
                ),
            }
        )

    res = run_bass_kernel_spmd(_COMPILED, in_maps, core_ids=list(range(N_CORES)))

    out = np.empty((B, L, D), np.float32)
    for c in range(N_CORES):
        bp, t = c // 4, c % 4
        for bi in range(BPC):
            out[BPC * bp + bi, :, HPC * DH * t : HPC * DH * (t + 1)] = res.results[
                c
            ]["out_r"][bi]
    return out


if __name__ == "__main__":
    rng = np.random.default_rng(0)
    x = rng.standard_normal((B, L, D)).astype(np.float32)
    k = (rng.random((H, D, D)) / D).astype(np.float32)
    q = (rng.random((H, D, D)) / D).astype(np.float32)
    v = (rng.random((H, DH, D)) / D).astype(np.float32)
    o = kernel(x=x, k=k, q=q, v=v)
    print("out", o.shape, o.dtype)
